# revision 47
# baseline (speedup 1.0000x reference)
"""ExclusiveSelfAttention TRN2 kernel: head-sharded tensor parallel over 8 NeuronCores.

Sharding: 16 heads / 8 cores = 2 heads (128 channels) per core.
Each core computes q/k/v projections for its 2 heads (full sequence),
attention + per-position Gram-Schmidt exclusion (head-local), and a
partial output projection (contraction over its 128 channels).
The host sums the 8 partials and adds the output bias.

Design notes (cost model: matmul time = moving-cols only, M/K free):
- Scores are computed transposed (e^T[j, i] tiles) so softmax-exp reads
  PSUM directly on the ACT engine.
- PV runs in the NATURAL orientation: out[i-tile(128), 65] accumulating
  over j with N=65 moving cols (v_h | ones), halving PV matmul time vs
  the transposed form, and a ones column gives sumexp Z per position.
- With PV output position-major, every exclusion scalar (v.v, o.v, 1/Z)
  is per-partition: the whole Gram-Schmidt step runs on DVE with
  tensor_tensor_reduce + tensor_scalar, no PE broadcasts.
- v-natural and o_f^T are produced by cheap PE transpose matmuls
  (identity fed from host), not DRAM round-trips.
"""

import sys

if '/opt/trn_rl_repo' not in sys.path:
    sys.path.insert(0, '/opt/trn_rl_repo')

import os
import numpy as np
import ml_dtypes

KN_ETBUFS = int(os.environ.get("KN_ETBUFS", "26"))
KN_XTV = int(os.environ.get("KN_XTV", "1"))
KN_ACTSTG = os.environ.get("KN_ACTSTG", "3:1")  # e.g. "2:1,3:01"
KN_VPRIO = int(os.environ.get("KN_VPRIO", "1950"))
KN_TILES = os.environ.get("KN_TILES", "2222")  # "332" or "2222"
KN_BIGW = 390 if KN_TILES == "332" else 260
KN_UCOPY = int(os.environ.get("KN_UCOPY", "1"))
KN_SC1B = int(os.environ.get("KN_SC1B", "2"))
KN_XTB = int(os.environ.get("KN_XTB", "4"))
KN_SCOFF = int(os.environ.get("KN_SCOFF", "3000"))
KN_PVOFF = int(os.environ.get("KN_PVOFF", "2200"))
KN_OSTGB = int(os.environ.get("KN_OSTGB", "3"))

import concourse.bass as bass
import concourse.mybir as mybir
import concourse.tile as tile
from concourse.tile import add_dep_helper
from concourse.bass_utils import run_bass_kernel_spmd

F32 = mybir.dt.float32
BF16 = mybir.dt.bfloat16
AF = mybir.ActivationFunctionType
ALU = mybir.AluOpType

B, S, D = 2, 2048, 1024
BS = B * S                    # 4096 combined (b, s) rows
HD = 64                       # head dim
E_LOC = 128                   # channels per core (2 heads)
N_CORES = 8
EPS = 1e-8
INV_SQRT_HD = 0.125

_ENGINE_TO_NC = {"PE": "tensor", "DVE": "vector", "Activation": "scalar",
                 "Pool": "gpsimd", "SP": "sync"}


def _make_nop(nc, engine):
    eng = getattr(nc, _ENGINE_TO_NC[str(engine).split(".")[-1]])
    r = eng.nop(nofuse=True, hint="waitsplit")
    ins = r.ins if hasattr(r, "ins") else r
    for blk in nc.main_func.blocks:
        insns = blk.instructions
        for i, x in enumerate(insns):
            if x.name == ins.name:
                del insns[i]
                blk.instructions = insns
                return ins
    raise RuntimeError("freshly created nop not found")


def split_waits(nc, limit=1):
    """Walrus codegen only encodes one sync-wait per instruction here; move
    excess waits onto preceding same-engine NOPs (same-engine program order
    makes this semantics-preserving)."""
    for blk in nc.main_func.blocks:
        ins_list = blk.instructions
        out, changed = [], False
        for ins in ins_list:
            si = ins.sync_info
            if si is not None and len(si.on_wait) > limit:
                waits = list(si.on_wait)
                extra, keep = waits[:-limit], waits[-limit:]
                for w in extra:
                    nop = _make_nop(nc, ins.engine)
                    nop.sync_info = mybir.SyncInfo(on_wait=[w], on_update=[])
                    out.append(nop)
                ins.sync_info = mybir.SyncInfo(on_wait=keep, on_update=list(si.on_update))
                changed = True
            out.append(ins)
        if changed:
            blk.instructions = out
    return nc


def build_program():
    nc = bass.Bass()

    xT_d = nc.declare_dram_parameter("xT", [D, BS], BF16, isOutput=False)
    wqT_d = nc.declare_dram_parameter("wqT", [E_LOC, D], BF16, isOutput=False)
    wkT_d = nc.declare_dram_parameter("wkT", [E_LOC, D], BF16, isOutput=False)
    wvT_d = nc.declare_dram_parameter("wvT", [E_LOC, D], BF16, isOutput=False)
    bq_d = nc.declare_dram_parameter("bq", [E_LOC], F32, isOutput=False)
    bk_d = nc.declare_dram_parameter("bk", [E_LOC], F32, isOutput=False)
    bv_d = nc.declare_dram_parameter("bv", [E_LOC], F32, isOutput=False)
    woT_d = nc.declare_dram_parameter("woT", [E_LOC, D], BF16, isOutput=False)
    ident_d = nc.declare_dram_parameter("ident", [128, 128], BF16, isOutput=False)
    part_d = nc.declare_dram_parameter("partial", [BS, D], BF16, isOutput=True)

    with tile.TileContext(nc) as tc:
        import contextlib
        with contextlib.ExitStack() as ctx:
            const = ctx.enter_context(tc.tile_pool(name="const", bufs=1))
            xt_pool = ctx.enter_context(tc.tile_pool(name="xt", bufs=KN_XTB))
            persist = ctx.enter_context(tc.tile_pool(name="persist", bufs=1))
            et_pool = ctx.enter_context(tc.tile_pool(name="et", bufs=30))
            vstg_pool = ctx.enter_context(tc.tile_pool(name="vstg", bufs=2))
            ofn_pool = ctx.enter_context(tc.tile_pool(name="ofn", bufs=6))
            oft_pool = ctx.enter_context(tc.tile_pool(name="oft", bufs=6))
            sc_pool = ctx.enter_context(tc.tile_pool(name="scr", bufs=8))
            sc1_pool = ctx.enter_context(tc.tile_pool(name="scr1", bufs=3))
            ostg_pool = ctx.enter_context(tc.tile_pool(name="ostg", bufs=KN_OSTGB))
            ps_sc = ctx.enter_context(tc.tile_pool(name="ps_sc", bufs=2, space="PSUM"))
            ps_gen = ctx.enter_context(tc.tile_pool(name="ps_gen", bufs=2, space="PSUM"))

            # ---- PE clock warmup: the cost model ramps the PE from
            # 0.65->2.4GHz only after 3us of continuous busy; burn the
            # DMA-feed dead time at the start on dummy matmuls so the real
            # projections run at full clock.
            with tc.high_priority(offset=9000):
                warm_a = const.tile([1, 1], BF16, tag="warm_a")
                nc.vector.memset(warm_a, 0.5)
                warm_b = const.tile([1, 512], BF16, tag="warm_b")
                nc.vector.memset(warm_b, 0.5)
                for wi in range(KN_WARM):
                    wps = ps_sc.tile([128, 1024], F32, tag="sc")
                    nc.tensor.matmul(wps[0:1, 0:512], warm_a, warm_b,
                                     start=True, stop=True)

            # ---- early x tile, then weights/constants ----
            def load_x(b, sh):
                scols = slice(b * S + sh * 512, b * S + (sh + 1) * 512)
                xt = xt_pool.tile([128, 8, 512], BF16, tag="xt")
                for kt2 in range(4):
                    nc.sync.dma_start(
                        out=xt[:, 2 * kt2:2 * kt2 + 2, :],
                        in_=xT_d[:, scols].rearrange("(kt p) s -> p kt s", kt=8)
                        [:, 2 * kt2:2 * kt2 + 2, :])
                return xt

            wsb = {}
            bsb = {}
            for name, wd, bd in (("q", wqT_d, bq_d), ("k", wkT_d, bk_d),
                                 ("v", wvT_d, bv_d)):
                t = const.tile([128, 8, E_LOC], BF16, tag=f"w{name}")
                if name != "v":
                    nc.sync.dma_start(
                        out=t, in_=wd[:, :].rearrange("p (kt e) -> p kt e", kt=8))
                wsb[name] = t
                tb = const.tile([128, 1], F32, tag=f"b{name}")
                nc.sync.dma_start(out=tb,
                                  in_=bd[:].rearrange("(p one) -> p one", one=1))
                bsb[name] = tb
            xts = {}

            # ---- persistent activations ----
            qT = persist.tile([128, BS], BF16, tag="qT")       # [e_loc, b*s]
            kT = persist.tile([128, BS], BF16, tag="kT")
            # natural v: vn[p, idx, c]: position = (idx%16)*128 + p of batch
            # idx//16; c: 0:64 = head0 v, 64 = 1.0, 65:129 = head1 v, 129 = 1.0
            vn = persist.tile([128, 32, 130], BF16, tag="vn")
            nc.vector.memset(
                vn.rearrange("p n (two c) -> p n two c", two=2)[:, :, :, 64:65], 1.0)

            ident = const.tile([128, 128], BF16, tag="ident")
            wo_sb = const.tile([128, D], BF16, tag="wo")

            def proj(name, xt):
                psp = ps_gen.tile([128, 512], F32, tag="prj")
                for kt in range(8):
                    nc.tensor.matmul(psp, wsb[name][:, kt, :], xt[:, kt, :],
                                     start=(kt == 0), stop=(kt == 7))
                return psp

            def emit_qk(b):
                for sh, what in ((0, "qk"), (1, "qk"), (2, "k"), (3, "k")):
                    if (b, sh) not in xts:
                        xts[(b, sh)] = load_x(b, sh)
                    scols = slice(b * S + sh * 512, b * S + (sh + 1) * 512)
                    if "q" in what:
                        nc.vector.tensor_scalar(out=qT[:, scols],
                                                in0=proj("q", xts[(b, sh)]),
                                                scalar1=bsb["q"], scalar2=None,
                                                op0=ALU.add)
                    nc.vector.tensor_scalar(out=kT[:, scols],
                                            in0=proj("k", xts[(b, sh)]),
                                            scalar1=bsb["k"], scalar2=None,
                                            op0=ALU.add)
                for sh in (2, 3):
                    scols = slice(b * S + sh * 512, b * S + (sh + 1) * 512)
                    nc.vector.tensor_scalar(out=qT[:, scols],
                                            in0=proj("q", xts[(b, sh)]),
                                            scalar1=bsb["q"], scalar2=None,
                                            op0=ALU.add)

            def emit_v(b):
                for sh in range(4):
                    vstg = vstg_pool.tile([128, 512], BF16, tag="vstg")
                    nc.vector.tensor_scalar(out=vstg, in0=proj("v", xts[(b, sh)]),
                                            scalar1=bsb["v"], scalar2=None,
                                            op0=ALU.add)
                    for t4 in range(4):
                        idx = b * 16 + sh * 4 + t4
                        ptr = ps_gen.tile([128, 128], BF16, tag="prj")
                        nc.tensor.transpose(ptr, vstg[:, t4 * 128:(t4 + 1) * 128],
                                            ident)
                        nc.vector.tensor_copy(
                            vn[:, idx].rearrange("p (two c) -> p two c", two=2)
                            [:, :, 0:64],
                            ptr.rearrange("p (two c) -> p two c", two=2))

            emit_qk(0)
            nc.sync.dma_start(
                out=wsb["v"],
                in_=wvT_d[:, :].rearrange("p (kt e) -> p kt e", kt=8))
            nc.sync.dma_start(out=ident, in_=ident_d[:, :])
            nc.sync.dma_start(out=wo_sb, in_=woT_d[:, :])
            emit_v(0)
            with tc.high_priority(offset=1200):
                emit_qk(1)
            emit_v(1)

            # ---- phase 2: attention + exclusion + out-proj per (b, ih) ----
            def emit_group(b, ih, act_stg):
                i0 = b * S + ih * 1024          # global i offset in [0, BS)
                et = []
                for jt in range(16):
                    jcol = slice(b * S + jt * 128, b * S + (jt + 1) * 128)
                    e_t = et_pool.tile([128, 2, 1024], BF16, tag="et")
                    for h in range(2):
                        hp = slice(h * 64, (h + 1) * 64)
                        pst = ps_sc.tile([128, 1024], F32, tag="sc")
                        with tc.high_priority(offset=KN_SCOFF):
                            for s2 in range(2):
                                icols = slice(i0 + s2 * 512, i0 + (s2 + 1) * 512)
                                nc.tensor.matmul(
                                    pst[:, s2 * 512:(s2 + 1) * 512],
                                    kT[hp, jcol], qT[hp, icols],
                                    start=True, stop=True,
                                    tile_position=(h * 64, 0))
                        nc.scalar.activation(e_t[:, h, :], pst, AF.Exp,
                                             bias=0.0, scale=INV_SQRT_HD)
                    et.append(e_t)

                tile_plan = (((0, 1, 2), (3, 4, 5), (6, 7))
                             if KN_TILES == "332" else
                             ((0, 1), (2, 3), (4, 5), (6, 7)))
                for L in tile_plan:
                    nL = len(L)
                    ii = [b * 16 + ih * 8 + it for it in L]
                    pv = ps_gen.tile([128, 390], F32, tag="pv", bufs=2)
                    firsts = []
                    with tc.high_priority(offset=KN_PVOFF):
                        for jt in range(16):
                            for h in range(2):
                                for li in range(nL):
                                    c0 = li * 130 + h * 65
                                    m = nc.tensor.matmul(
                                        pv[:, c0:c0 + 65],
                                        et[jt][:, h, L[li] * 128:
                                               (L[li] + 1) * 128],
                                        vn[:, b * 16 + jt, h * 65:h * 65 + 65],
                                        start=(jt == 0 and h == 0 and li == 0),
                                        stop=(jt == 15 and h == 1 and
                                              li == nL - 1))
                                    if jt == 0:
                                        firsts.append(
                                            m.ins if hasattr(m, "ins") else m)
                        for f in firsts[1:]:
                            add_dep_helper(f, firsts[0],
                                           reason="PSUM zero-region order")
                    W = nL * 130
                    nq = 2 * nL
                    vnt2 = vn[:, ii[0]:ii[0] + nL, :]     # [128, nL, 130]
                    with tc.high_priority(offset=KN_PVOFF):
                        # Free the PSUM tile immediately: one copy is the only
                        # pv reader, so the pv ring recycles right after each
                        # chain and later chains trail the exps tightly.
                        if KN_UCOPY:
                            uc = sc1_pool.tile([128, KN_BIGW], F32, tag="uc", bufs=KN_SC1B)
                            nc.vector.tensor_copy(uc[:, 0:W], pv[:, 0:W])
                        else:
                            uc = pv
                        # v.v + eps and u.v sums batched over nL i-tiles x 2
                        # heads ([128,nq] lanes); ones columns add +1 / +Z,
                        # removed via the fused (x - zz) ops below.
                        scr = sc1_pool.tile([128, KN_BIGW], BF16, tag="s1", bufs=KN_SC1B)
                        nc.vector.tensor_tensor(out=scr[:, 0:W], in0=vnt2,
                                                in1=vnt2, op=ALU.mult)
                        vv = sc_pool.tile([128, 8], F32, tag="vv")
                        nc.vector.tensor_reduce(
                            out=vv[:, 0:nq],
                            in_=scr[:, 0:W].rearrange(
                                "p (l c) -> p l c", c=65),
                            axis=mybir.AxisListType.X, op=ALU.add)
                        vve = sc_pool.tile([128, 8], F32, tag="vve")
                        nc.vector.tensor_scalar(out=vve[:, 0:nq],
                                                in0=vv[:, 0:nq],
                                                scalar1=EPS - 1.0,
                                                scalar2=None, op0=ALU.add)
                        rv = sc_pool.tile([128, 8], F32, tag="rv")
                        nc.vector.reciprocal(rv[:, 0:nq], vve[:, 0:nq])
                        scr2 = sc1_pool.tile([128, KN_BIGW], F32, tag="s2", bufs=KN_SC1B)
                        nc.vector.tensor_tensor(out=scr2[:, 0:W],
                                                in0=uc[:, 0:W],
                                                in1=vnt2, op=ALU.mult)
                        uvz = sc_pool.tile([128, 8], F32, tag="uvz")
                        nc.vector.tensor_reduce(
                            out=uvz[:, 0:nq],
                            in_=scr2[:, 0:W].rearrange(
                                "p (l c) -> p l c", c=65),
                            axis=mybir.AxisListType.X, op=ALU.add)
                        zz = uc[:, 0:W].rearrange("p (l c) -> p l c", c=65)[
                            :, :, 64:65].rearrange("p a b -> p (a b)")
                        r = sc_pool.tile([128, 8], F32, tag="r")
                        nc.vector.reciprocal(r[:, 0:nq], zz)
                        a = sc_pool.tile([128, 8], F32, tag="a")
                        for q in range(nq):
                            # a = (uv+Z - Z) * 1/(v.v+eps), fused per lane
                            nc.vector.tensor_scalar(
                                out=a[:, q:q + 1], in0=uvz[:, q:q + 1],
                                scalar1=zz[:, q:q + 1], op0=ALU.subtract,
                                scalar2=rv[:, q:q + 1], op1=ALU.mult)
                        ofns = []
                        for li in range(nL):
                            ofn = ofn_pool.tile([128, 128], BF16, tag="ofn")
                            ofns.append(ofn)
                            for h in range(2):
                                q = li * 2 + h
                                c0 = li * 130 + h * 65
                                av = sc1_pool.tile([128, 64], BF16, tag="av", bufs=8)
                                nc.gpsimd.tensor_scalar(
                                    out=av,
                                    in0=vn[:, ii[li], h * 65:h * 65 + 64],
                                    scalar1=a[:, q:q + 1], scalar2=None,
                                    op0=ALU.mult)
                                gg = sc1_pool.tile([128, 64], F32, tag="gg", bufs=8)
                                nc.gpsimd.tensor_tensor(
                                    out=gg, in0=uc[:, c0:c0 + 64], in1=av,
                                    op=ALU.subtract)
                                nc.gpsimd.tensor_scalar(
                                    out=ofns[li][:, h * 64:(h + 1) * 64],
                                    in0=gg,
                                    scalar1=r[:, q:q + 1], scalar2=None,
                                    op0=ALU.mult)
                    for li in range(nL):
                        it = L[li]
                        with tc.high_priority(offset=KN_PVOFF):
                            ptr = ps_gen.tile([128, 128], BF16, tag="prj")
                            nc.tensor.transpose(ptr, ofns[li], ident)
                            ofT = oft_pool.tile([128, 128], BF16, tag="ofT")
                            nc.vector.tensor_copy(ofT, ptr)
                            for eb in range(2):
                                lastg = (b == 1 and ih == 1)
                                if lastg:
                                    ps_o2 = ps_sc.tile([128, 512], F32,
                                                       tag="sc", name="pso2s")
                                else:
                                    ps_o2 = ps_gen.tile([128, 512], F32,
                                                        tag="prj", name="pso2g")
                                nc.tensor.matmul(ps_o2, ofT,
                                                 wo_sb[:, eb * 512:(eb + 1) * 512],
                                                 start=True, stop=True)
                                stg = ostg_pool.tile([128, 512], BF16, tag="ostg")
                                if eb in act_stg:
                                    nc.scalar.copy(stg, ps_o2)
                                else:
                                    nc.vector.tensor_copy(stg, ps_o2)
                                nc.sync.dma_start(
                                    out=part_d[i0 + it * 128:i0 + (it + 1) * 128,
                                               eb * 512:(eb + 1) * 512],
                                    in_=stg)

            stg_map = {0: (), 1: (), 2: (), 3: ()}
            for part in KN_ACTSTG.split(","):
                if not part:
                    continue
                gi, ebs = part.split(":")
                stg_map[int(gi)] = tuple(int(c) for c in ebs)
            emit_group(0, 0, stg_map[0])
            emit_group(0, 1, stg_map[1])
            emit_group(1, 0, stg_map[2])
            emit_group(1, 1, stg_map[3])

    split_waits(nc)
    return nc


_CACHE = {}


def kernel(x, wq, bq, wk, bk, wv, bv, wo, bo):
    x = np.ascontiguousarray(np.asarray(x, dtype=np.float32))
    wq, wk, wv, wo = (np.asarray(w, dtype=np.float32) for w in (wq, wk, wv, wo))
    bq, bk, bv, bo = (np.asarray(v, dtype=np.float32) for v in (bq, bk, bv, bo))

    if "nc" not in _CACHE:
        _CACHE["nc"] = build_program()
    nc = _CACHE["nc"]

    xT = np.ascontiguousarray(x.reshape(BS, D).T).astype(ml_dtypes.bfloat16)
    ident = np.eye(128, dtype=ml_dtypes.bfloat16)

    def wshuf(w, cs):
        # sbuf layout [p, kt, e]: value = w[cs].T[kt*128+p, e]
        t = np.asarray(w[cs, :].T, np.float32).reshape(8, 128, E_LOC)
        return np.ascontiguousarray(
            t.transpose(1, 0, 2).reshape(128, 8 * E_LOC)).astype(
                ml_dtypes.bfloat16)

    in_maps = []
    for g in range(N_CORES):
        cs = slice(g * E_LOC, (g + 1) * E_LOC)
        in_maps.append({
            "xT": xT,
            "wqT": wshuf(wq, cs),
            "wkT": wshuf(wk, cs),
            "wvT": wshuf(wv, cs),
            "bq": np.ascontiguousarray(bq[cs]),
            "bk": np.ascontiguousarray(bk[cs]),
            "bv": np.ascontiguousarray(bv[cs]),
            "woT": np.ascontiguousarray(wo[:, cs].T).astype(ml_dtypes.bfloat16),
            "ident": ident,
        })

    res = run_bass_kernel_spmd(nc, in_maps, list(range(N_CORES)))
    out = np.zeros((BS, D), np.float32)
    for g in range(N_CORES):
        out += np.asarray(res.results[g]["partial"], np.float32)
    out += bo[None, :]
    return out.reshape(B, S, D)
